# revision 21
# baseline (speedup 1.0000x reference)
"""NCC loss (VoxelMorph-style, 9^3 box window) on 8 Trainium2 NeuronCores.

Strategy: data-parallel over the depth axis. Each core gets a 16-slice output
chunk plus a 4-slice halo on each side (zero-padded at volume edges), for both
batch elements. Per core:
  products I*I, J*J, I*J (DVE/ACT)
  D-axis 9-window sum: tree adds on DVE (win9 = three win3 sums composed)
  H-axis then W-axis 9-window sums: two chained matmuls per slice against a
    banded all-ones matrix; lhsT = data (stationary) so each matmul both
    box-sums one axis and transposes the tile, landing back in [H', W'] layout
  NCC elementwise math, fused reduce into per-partition partial sums
Host sums the 8x128 partials and forms 1 - total/N.
"""

from contextlib import ExitStack

import numpy as np

WIN = 9
PAD = WIN // 2  # 4
B = 2
D = 128
H = 128
W = 128
NCORES = 8
D_OUT = D // NCORES  # 16
D_IN = D_OUT + 2 * PAD  # 24
EPS = 1e-6
WIN_SIZE = 729.0
N_TOTAL = float(B * D * H * W)

_CACHE = {}


def _split_multiwaits(nc):
    """Walrus in this env encodes at most ONE sync-wait per instruction.
    Hoist extra waits onto standalone EventSemaphore insts just before."""
    from concourse import mybir

    n = 0
    for fn in nc.m.functions:
        for bb in fn.blocks:
            il = bb.instructions
            out = []
            for inst in il:
                si = inst.sync_info
                if si is not None and si.on_wait and len(si.on_wait) > 1:
                    waits = list(si.on_wait)
                    for w in waits[:-1]:
                        ev = mybir.InstEventSemaphore(
                            name=f"EVW-{n}", ins=[], outs=[])
                        n += 1
                        ev.engine = inst.engine
                        ev.sync_info = mybir.SyncInfo(on_wait=[w],
                                                      on_update=[])
                        out.append(ev)
                    inst.sync_info = mybir.SyncInfo(
                        on_wait=[waits[-1]], on_update=list(si.on_update))
                out.append(inst)
            il[:] = out
    return n


def _build_nc():
    import concourse.bass as bass
    import concourse.tile as tile
    from concourse import mybir

    f32 = mybir.dt.float32
    bf16 = mybir.dt.bfloat16
    Alu = mybir.AluOpType
    Act = mybir.ActivationFunctionType

    nc = bass.Bass()
    I_ext = nc.declare_dram_parameter("I", [B, D_IN, H, W], f32, isOutput=False)
    J_ext = nc.declare_dram_parameter("J", [B, D_IN, H, W], f32, isOutput=False)
    BB_ext = nc.declare_dram_parameter("BB", [H, H], bf16, isOutput=False)
    out_ext = nc.declare_dram_parameter("partials", [128, 1], f32, isOutput=True)

    G = 8  # slices per PSUM copy group

    with tile.TileContext(nc) as tc, ExitStack() as ctx:
        singles = ctx.enter_context(tc.tile_pool(name="singles", bufs=1))
        src = ctx.enter_context(tc.tile_pool(name="src", bufs=1))
        ldp = ctx.enter_context(tc.tile_pool(name="ldp", bufs=2))
        dtmp = ctx.enter_context(tc.tile_pool(name="dtmp", bufs=2))
        boxp = ctx.enter_context(tc.tile_pool(name="boxp", bufs=1))
        yp = ctx.enter_context(tc.tile_pool(name="yp", bufs=2))
        qp = ctx.enter_context(tc.tile_pool(name="qp", bufs=1))
        pp = ctx.enter_context(tc.tile_pool(name="pp", bufs=2))
        psum = ctx.enter_context(tc.tile_pool(name="psum", bufs=2, space="PSUM"))

        BBt = singles.tile([H, H], bf16)
        nc.sync.dma_start(out=BBt, in_=BB_ext[:, :])

        parts = []
        for b in range(B):
            tI = ldp.tile([H, D_IN, W], f32, tag="tI")
            tJ = ldp.tile([H, D_IN, W], f32, tag="tJ")
            nc.sync.dma_start(out=tI, in_=I_ext[b].rearrange("d h w -> h d w"))
            nc.sync.dma_start(out=tJ, in_=J_ext[b].rearrange("d h w -> h d w"))

            # bf16 working copies (box-sum path runs in bf16)
            tIb = src.tile([H, D_IN, W], bf16, tag="tIb")
            tJb = src.tile([H, D_IN, W], bf16, tag="tJb")
            nc.vector.tensor_copy(out=tIb, in_=tI)
            nc.vector.tensor_copy(out=tJb, in_=tJ)
            tI2 = src.tile([H, D_IN, W], bf16, tag="tI2")
            tJ2 = src.tile([H, D_IN, W], bf16, tag="tJ2")
            tIJ = src.tile([H, D_IN, W], bf16, tag="tIJ")
            nc.scalar.activation(out=tI2, in_=tI, func=Act.Square)
            nc.scalar.activation(out=tJ2, in_=tJ, func=Act.Square)
            nc.vector.tensor_mul(out=tIJ, in0=tIb, in1=tJb)

            boxes = {}
            for name, V in (("I", tIb), ("J", tJb), ("I2", tI2), ("J2", tJ2),
                            ("IJ", tIJ)):
                # D-axis win3 on DVE (bf16 2x); the win3-of-win3 completion
                # happens inside mm1's PSUM accumulation below.
                a = dtmp.tile([H, D_IN - 2, W], bf16, tag="a")
                t3 = dtmp.tile([H, D_IN - 2, W], bf16, tag="t3")
                nc.vector.tensor_add(out=a, in0=V[:, 0:22, :], in1=V[:, 1:23, :])
                nc.vector.tensor_add(out=t3, in0=a, in1=V[:, 2:24, :])

                # mm1 (x3, accum): out1[w,h'] = sum_m sum_h t3[h,o+m,w]*BB[h,h']
                #   -> H-sum + D-win9 completion, transposed into [W, H']
                # mm2: out2[h',w'] = sum_w y[w,h']*BB[w,w']  (W-sum, back)
                # PSUM drained in groups of G slices per copy.
                box = boxp.tile([H, D_OUT, W], bf16, tag="box" + name)
                for g in range(0, D_OUT, G):
                    pg1 = psum.tile([128, G, 128], f32, tag="pg1")
                    for k in range(G):
                        for mi, m in enumerate((0, 3, 6)):
                            nc.tensor.matmul(out=pg1[:, k, :],
                                             lhsT=t3[:, g + k + m, :],
                                             rhs=BBt, start=(mi == 0),
                                             stop=(mi == 2))
                    y = yp.tile([128, G, 128], bf16, tag="y")
                    nc.any.tensor_copy(out=y, in_=pg1)
                    pg2 = psum.tile([128, G, 128], f32, tag="pg2")
                    for k in range(G):
                        nc.tensor.matmul(out=pg2[:, k, :], lhsT=y[:, k, :],
                                         rhs=BBt, start=True, stop=True)
                    nc.any.tensor_copy(out=box[:, g:g + G, :], in_=pg2)
                boxes[name] = box

            SI, SJ, SI2, SJ2, SIJ = (boxes[k] for k in
                                     ("I", "J", "I2", "J2", "IJ"))
            # NCC math (all [128, 16, 128] bf16 for DVE 2x; in-place reuse).
            qI = qp.tile([H, D_OUT, W], bf16, tag="qI")
            qJ = qp.tile([H, D_OUT, W], bf16, tag="qJ")
            # qI = (SI/27)^2 = SI^2/729 ;  Iv = SI2 - qI
            nc.scalar.activation(out=qI, in_=SI, func=Act.Square,
                                 scale=1.0 / 27.0)
            nc.scalar.activation(out=qJ, in_=SJ, func=Act.Square,
                                 scale=1.0 / 27.0)
            nc.vector.scalar_tensor_tensor(out=qI, in0=qI, scalar=-1.0,
                                           in1=SI2, op0=Alu.mult, op1=Alu.add)
            nc.vector.scalar_tensor_tensor(out=qJ, in0=qJ, scalar=-1.0,
                                           in1=SJ2, op0=Alu.mult, op1=Alu.add)
            # qJ = max(Jv, eps); qI = max(Iv, eps) * qJ
            nc.vector.tensor_scalar_max(out=qJ, in0=qJ, scalar1=EPS)
            nc.vector.scalar_tensor_tensor(out=qI, in0=qI, scalar=EPS,
                                           in1=qJ, op0=Alu.max, op1=Alu.mult)
            # rsqrt via ACT: qI = exp(-0.5 * ln(V))
            nc.scalar.activation(out=qJ, in_=qI, func=Act.Ln)
            nc.scalar.activation(out=qI, in_=qJ, func=Act.Exp, scale=-0.5)
            # SI = SI*SJ ; SI = IJ_sum - SI/729  (= cross)
            nc.vector.tensor_mul(out=SI, in0=SI, in1=SJ)
            nc.vector.scalar_tensor_tensor(out=SI, in0=SI,
                                           scalar=-1.0 / WIN_SIZE, in1=SIJ,
                                           op0=Alu.mult, op1=Alu.add)
            # cc = SI * qI ; partial[p] = sum_free(cc)
            nc.vector.tensor_mul(out=SJ, in0=SI, in1=qI)
            part = pp.tile([128, 1], f32, tag="part")
            nc.vector.tensor_reduce(out=part, in_=SJ,
                                    axis=mybir.AxisListType.XY, op=Alu.add)
            parts.append(part)

        total = pp.tile([128, 1], f32, tag="total")
        nc.vector.tensor_add(out=total, in0=parts[0], in1=parts[1])
        nc.sync.dma_start(out=out_ext[:, :], in_=total)

    return nc


def _get_nc(split=True):
    if "nc" not in _CACHE:
        _CACHE["nc"] = _build_nc()
    if split and not _CACHE.get("split"):
        _split_multiwaits(_CACHE["nc"])
        _CACHE["split"] = True
    return _CACHE["nc"]


def _shards(y_true, y_pred):
    yt = np.ascontiguousarray(
        np.asarray(y_true, dtype=np.float32).reshape(B, D, H, W))
    yp = np.ascontiguousarray(
        np.asarray(y_pred, dtype=np.float32).reshape(B, D, H, W))
    pt = np.zeros((B, D + 2 * PAD, H, W), dtype=np.float32)
    pp = np.zeros((B, D + 2 * PAD, H, W), dtype=np.float32)
    pt[:, PAD:PAD + D] = yt
    pp[:, PAD:PAD + D] = yp

    import ml_dtypes
    BB = np.zeros((H, H), dtype=np.float32)
    for i in range(H):
        BB[i, max(0, i - PAD):min(H, i + PAD + 1)] = 1.0
    BB_bf16 = BB.astype(ml_dtypes.bfloat16)

    in_maps = []
    for c in range(NCORES):
        lo = c * D_OUT
        in_maps.append({
            "I": np.ascontiguousarray(pt[:, lo:lo + D_IN]),
            "J": np.ascontiguousarray(pp[:, lo:lo + D_IN]),
            "BB": BB_bf16,
        })
    return in_maps


def run(y_true, y_pred, trace=False):
    from concourse.bass_utils import run_bass_kernel_spmd

    nc = _get_nc()
    in_maps = _shards(y_true, y_pred)
    res = run_bass_kernel_spmd(nc, in_maps, list(range(NCORES)), trace=trace)
    total = 0.0
    for r in res.results:
        total += float(np.asarray(r["partials"], dtype=np.float64).sum())
    loss = np.float32(1.0 - total / N_TOTAL)
    return np.array(loss, dtype=np.float32), res


def kernel(y_true, y_pred):
    loss, _ = run(y_true, y_pred, trace=False)
    return loss


# revision 23
# speedup vs baseline: 1.3904x; 1.3904x over previous
"""NCC loss (VoxelMorph-style, 9^3 box window) on 8 Trainium2 NeuronCores.

Strategy: data-parallel over the depth axis. Each core gets a 16-slice output
chunk plus a 4-slice halo on each side (zero-padded at volume edges), for both
batch elements. Per core (box-sum pipeline in bf16, NCC mostly bf16):
  products I*I, J*J (ACT Square), I*J (DVE)
  D-axis win3 stage on DVE; win9 completion via 3 accumulating matmuls
  H-axis then W-axis 9-window sums: chained matmuls against a banded ones
    matrix; lhsT = data (stationary) so each matmul box-sums one axis AND
    transposes, landing back in [H', W'] layout with no transpose insts
  per-G-slice groups: PSUM drain + NCC elementwise math + partial reduction,
    pipelined across groups/volumes to keep DVE/ACT/PE all busy
Host sums the 8x128 partials and forms 1 - total/N.
"""

from contextlib import ExitStack

import numpy as np

WIN = 9
PAD = WIN // 2  # 4
B = 2
D = 128
H = 128
W = 128
NCORES = 8
D_OUT = D // NCORES  # 16
D_IN = D_OUT + 2 * PAD  # 24
EPS = 1e-6
WIN_SIZE = 729.0
N_TOTAL = float(B * D * H * W)

_CACHE = {}


def _split_multiwaits(nc):
    """Walrus in this env encodes at most ONE sync-wait per instruction.
    Hoist extra waits onto standalone EventSemaphore insts just before."""
    from concourse import mybir

    n = 0
    for fn in nc.m.functions:
        for bb in fn.blocks:
            il = bb.instructions
            out = []
            for inst in il:
                si = inst.sync_info
                if si is not None and si.on_wait and len(si.on_wait) > 1:
                    waits = list(si.on_wait)
                    for w in waits[:-1]:
                        ev = mybir.InstEventSemaphore(
                            name=f"EVW-{n}", ins=[], outs=[])
                        n += 1
                        ev.engine = inst.engine
                        ev.sync_info = mybir.SyncInfo(on_wait=[w],
                                                      on_update=[])
                        out.append(ev)
                    inst.sync_info = mybir.SyncInfo(
                        on_wait=[waits[-1]], on_update=list(si.on_update))
                out.append(inst)
            il[:] = out
    return n


def _build_nc():
    import concourse.bass as bass
    import concourse.tile as tile
    from concourse import mybir

    f32 = mybir.dt.float32
    bf16 = mybir.dt.bfloat16
    Alu = mybir.AluOpType
    Act = mybir.ActivationFunctionType

    nc = bass.Bass()
    I_ext = nc.declare_dram_parameter("I", [B, D_IN, H, W], bf16,
                                      isOutput=False)
    J_ext = nc.declare_dram_parameter("J", [B, D_IN, H, W], bf16,
                                      isOutput=False)
    BB_ext = nc.declare_dram_parameter("BB", [H, H], bf16, isOutput=False)
    out_ext = nc.declare_dram_parameter("partials", [128, 1], f32,
                                        isOutput=True)

    G = 8  # slices per PSUM drain / NCC group
    NV = D_IN - 2  # 22 win3 slices
    VOLS = ("I", "J", "I2", "J2", "IJ")

    with tile.TileContext(nc) as tc, ExitStack() as ctx:
        singles = ctx.enter_context(tc.tile_pool(name="singles", bufs=1))
        src = ctx.enter_context(tc.tile_pool(name="src", bufs=2))
        dtmp = ctx.enter_context(tc.tile_pool(name="dtmp", bufs=2))
        boxp = ctx.enter_context(tc.tile_pool(name="boxp", bufs=2))
        yp = ctx.enter_context(tc.tile_pool(name="yp", bufs=2))
        qp = ctx.enter_context(tc.tile_pool(name="qp", bufs=2))
        pp = ctx.enter_context(tc.tile_pool(name="pp", bufs=8))
        psum = ctx.enter_context(tc.tile_pool(name="psum", bufs=2,
                                              space="PSUM"))

        BBt = singles.tile([H, H], bf16)
        nc.sync.dma_start(out=BBt, in_=BB_ext[:, :])

        parts = []
        for b in range(B):
            tIb = src.tile([H, D_IN, W], bf16, tag="tIb")
            tJb = src.tile([H, D_IN, W], bf16, tag="tJb")
            nc.sync.dma_start(out=tIb,
                              in_=I_ext[b].rearrange("d h w -> h d w"))
            nc.sync.dma_start(out=tJb,
                              in_=J_ext[b].rearrange("d h w -> h d w"))

            tI2 = src.tile([H, D_IN, W], bf16, tag="tI2")
            tJ2 = src.tile([H, D_IN, W], bf16, tag="tJ2")
            tIJ = src.tile([H, D_IN, W], bf16, tag="tIJ")
            nc.scalar.activation(out=tI2, in_=tIb, func=Act.Square)
            nc.scalar.activation(out=tJ2, in_=tJb, func=Act.Square)
            nc.vector.tensor_mul(out=tIJ, in0=tIb, in1=tJb)

            # D-axis win3 for all 5 volumes (bf16 2x adds on DVE)
            t3s = {}
            for name, V in (("I", tIb), ("J", tJb), ("I2", tI2),
                            ("J2", tJ2), ("IJ", tIJ)):
                a = dtmp.tile([H, NV, W], bf16, tag="a")
                t3 = dtmp.tile([H, NV, W], bf16, tag="t3" + name)
                nc.vector.tensor_add(out=a, in0=V[:, 0:NV, :],
                                     in1=V[:, 1:NV + 1, :])
                nc.vector.tensor_add(out=t3, in0=a, in1=V[:, 2:NV + 2, :])
                t3s[name] = t3

            # per-group: H+W matmul passes for all 5 vols, then NCC + reduce
            for g in range(0, D_OUT, G):
                boxg = {}
                for name in VOLS:
                    t3 = t3s[name]
                    pg1 = psum.tile([128, G, 128], f32, tag="pg1")
                    for k in range(G):
                        for mi, m in enumerate((0, 3, 6)):
                            nc.tensor.matmul(out=pg1[:, k, :],
                                             lhsT=t3[:, g + k + m, :],
                                             rhs=BBt, start=(mi == 0),
                                             stop=(mi == 2))
                    y = yp.tile([128, G, 128], bf16, tag="y")
                    nc.scalar.copy(out=y, in_=pg1)
                    pg2 = psum.tile([128, G, 128], f32, tag="pg2")
                    for k in range(G):
                        nc.tensor.matmul(out=pg2[:, k, :], lhsT=y[:, k, :],
                                         rhs=BBt, start=True, stop=True)
                    bx = boxp.tile([128, G, 128], bf16, tag="bx" + name)
                    nc.any.tensor_copy(out=bx, in_=pg2)
                    boxg[name] = bx

                SI, SJ, SI2, SJ2, SIJ = (boxg[k] for k in VOLS)
                qI = qp.tile([128, G, 128], bf16, tag="qI")
                qJ = qp.tile([128, G, 128], bf16, tag="qJ")
                # qI = (SI/27)^2 ; Iv = SI2 - qI (clamped); same for J
                nc.scalar.activation(out=qI, in_=SI, func=Act.Square,
                                     scale=1.0 / 27.0)
                nc.scalar.activation(out=qJ, in_=SJ, func=Act.Square,
                                     scale=1.0 / 27.0)
                nc.vector.tensor_sub(out=qI, in0=SI2, in1=qI)
                nc.vector.tensor_sub(out=qJ, in0=SJ2, in1=qJ)
                nc.vector.tensor_scalar_max(out=qI, in0=qI, scalar1=EPS)
                nc.vector.tensor_scalar_max(out=qJ, in0=qJ, scalar1=EPS)
                nc.vector.tensor_mul(out=qI, in0=qI, in1=qJ)
                # r = rsqrt(V)/729 = exp(-0.5 * ln(V * 729^2))  (ACT)
                nc.scalar.activation(out=qJ, in_=qI, func=Act.Ln,
                                     scale=WIN_SIZE * WIN_SIZE)
                nc.scalar.activation(out=qI, in_=qJ, func=Act.Exp,
                                     scale=-0.5)
                # C9 = 729*IJ_sum - I_sum*J_sum ; cc = C9 * r
                nc.vector.tensor_mul(out=qJ, in0=SI, in1=SJ)
                nc.vector.tensor_scalar_mul(out=SIJ, in0=SIJ,
                                            scalar1=WIN_SIZE)
                nc.vector.tensor_sub(out=qJ, in0=SIJ, in1=qJ)
                nc.vector.tensor_mul(out=qJ, in0=qJ, in1=qI)
                part = pp.tile([128, 1], f32, tag="part")
                nc.vector.tensor_reduce(out=part, in_=qJ,
                                        axis=mybir.AxisListType.XY,
                                        op=Alu.add)
                parts.append(part)

        # combine the 4 group partials
        t01 = pp.tile([128, 1], f32, tag="t01")
        t23 = pp.tile([128, 1], f32, tag="t23")
        total = pp.tile([128, 1], f32, tag="total")
        nc.vector.tensor_add(out=t01, in0=parts[0], in1=parts[1])
        nc.vector.tensor_add(out=t23, in0=parts[2], in1=parts[3])
        nc.vector.tensor_add(out=total, in0=t01, in1=t23)
        nc.sync.dma_start(out=out_ext[:, :], in_=total)

    return nc


def _get_nc(split=True):
    if "nc" not in _CACHE:
        _CACHE["nc"] = _build_nc()
    if split and not _CACHE.get("split"):
        _split_multiwaits(_CACHE["nc"])
        _CACHE["split"] = True
    return _CACHE["nc"]


def _shards(y_true, y_pred):
    import ml_dtypes

    yt = np.ascontiguousarray(
        np.asarray(y_true, dtype=np.float32).reshape(B, D, H, W))
    yp = np.ascontiguousarray(
        np.asarray(y_pred, dtype=np.float32).reshape(B, D, H, W))
    pt = np.zeros((B, D + 2 * PAD, H, W), dtype=ml_dtypes.bfloat16)
    pp = np.zeros((B, D + 2 * PAD, H, W), dtype=ml_dtypes.bfloat16)
    pt[:, PAD:PAD + D] = yt.astype(ml_dtypes.bfloat16)
    pp[:, PAD:PAD + D] = yp.astype(ml_dtypes.bfloat16)

    BB = np.zeros((H, H), dtype=np.float32)
    for i in range(H):
        BB[i, max(0, i - PAD):min(H, i + PAD + 1)] = 1.0
    BB_bf16 = BB.astype(ml_dtypes.bfloat16)

    in_maps = []
    for c in range(NCORES):
        lo = c * D_OUT
        in_maps.append({
            "I": np.ascontiguousarray(pt[:, lo:lo + D_IN]),
            "J": np.ascontiguousarray(pp[:, lo:lo + D_IN]),
            "BB": BB_bf16,
        })
    return in_maps


def run(y_true, y_pred, trace=False):
    from concourse.bass_utils import run_bass_kernel_spmd

    nc = _get_nc()
    in_maps = _shards(y_true, y_pred)
    res = run_bass_kernel_spmd(nc, in_maps, list(range(NCORES)), trace=trace)
    total = 0.0
    for r in res.results:
        total += float(np.asarray(r["partials"], dtype=np.float64).sum())
    loss = np.float32(1.0 - total / N_TOTAL)
    return np.array(loss, dtype=np.float32), res


def kernel(y_true, y_pred):
    loss, _ = run(y_true, y_pred, trace=False)
    return loss
